# revision 16
# baseline (speedup 1.0000x reference)
"""CoxPHLoss (segment_reduce) Trainium2 kernel, 8-core SPMD.

Strategy (per the sharding hint, adapted to a segment-sharded layout):
  - Shard the N=8M sample axis across the 8 cores by duration-bin range
    (core s owns bins [1250*s, 1250*(s+1))). The host-side sharding step
    lays each core's samples out as a dense, zero-padded
    [bins_per_shard, max_bin_count] matrix (a pure integer permutation --
    no floating point work happens on the host).
  - On device, each core computes its per-bin segment sums
    (sum exp(log_h), sum events, sum exp(2*log_h), sum exp(log_h)*event)
    as dense row reductions, fused into scalar-engine activations and
    vector-engine accumulating ops.
  - The [K] histograms are exchanged with a single AllGather; the
    suffix-cumsum (risk), base hazard, and final MSE contraction are
    computed replicated on every core (triangular-matrix matmuls on the
    tensor engine for the scans).
  - The MSE reduction is algebraically expanded so only bin-level
    aggregates are needed:
        mse*N = sum_k base_k^2 * S2_k - 2 * sum_k base_k * T_k + E
    with S2_k = sum exp(2*log_h), T_k = sum exp(log_h)*event, E = sum e.

Inputs are shipped to the device as bf16 (exact for events; for log_h the
rounding perturbs the final loss by ~1e-7 relative, far below the f32
noise floor of the reduction itself).

Everything is hardcoded for the nn_CoxPHLoss problem:
  N = 8_000_000 samples, K = 10_000 duration bins, 8 cores.
"""

import os
import numpy as np

N = 8_000_000
K = 10_000
NCORES = 8
BINS_PER_SHARD = K // NCORES          # 1250
R = 1280                              # padded rows (bins) per shard, 10 chunks of 128
NCHUNK = R // 128                     # 10
PAD_LH = -10000.0                     # exp(PAD_LH) == 0 (also after bf16 rounding)

# Set by the builder; test.py can read these after a traced run.
LAST_EXEC_TIME_NS = None
LAST_RESULTS = None
TRACE = bool(int(os.environ.get("KERNEL_TRACE", "0")))

_CACHE = {}


def _build_program(C: int):
    """Build + compile the SPMD Bass program for slot capacity C."""
    import concourse.bacc as bacc
    import concourse.mybir as mybir
    import concourse.tile as tile

    f32 = mybir.dt.float32
    bf16 = mybir.dt.bfloat16
    Alu = mybir.AluOpType
    Act = mybir.ActivationFunctionType
    Ax = mybir.AxisListType

    nc = bacc.Bacc("TRN2", target_bir_lowering=False, debug=False,
                   num_devices=NCORES)

    lh_d = nc.dram_tensor("lh_d", [R, C], bf16, kind="ExternalInput")
    e_d = nc.dram_tensor("e_d", [R, C], bf16, kind="ExternalInput")
    mse_d = nc.dram_tensor("mse_d", [1, 1], f32, kind="ExternalOutput")

    # chunk views: [128, NCHUNK, C], partition = row within a 128-row chunk
    lh_v = lh_d.ap().rearrange("(a p) w -> p a w", p=128)
    e_v = e_d.ap().rearrange("(a p) w -> p a w", p=128)

    # constant masks for the tensor-engine scans
    tril_inc_h = nc.inline_tensor(
        np.tril(np.ones((128, 128), np.float32)), name="tril_inc")
    tril_str_h = nc.inline_tensor(
        np.tril(np.ones((128, 128), np.float32), -1), name="tril_str")
    ones_h = nc.inline_tensor(np.ones((128, 1), np.float32), name="ones128")
    allones_h = nc.inline_tensor(np.ones((128, 128), np.float32), name="allones")

    DMA_BATCH = 2  # chunks per input DMA (~1 MiB per transfer in bf16)

    with tile.TileContext(nc) as tc:
        with (
            tc.tile_pool(name="io", bufs=1) as io_pool,
            tc.tile_pool(name="scr", bufs=4) as scr_pool,
            tc.tile_pool(name="small", bufs=1) as small_pool,
            tc.tile_pool(name="psum", bufs=1, space="PSUM") as psum_pool,
            tc.tile_pool(name="dram", bufs=1, space="DRAM") as dram_pool,
        ):
            # whole input resident in SBUF (bf16: 2*19.2KB per partition)
            lh_all = io_pool.tile([128, NCHUNK, C], bf16, tag="lha")
            e_all = io_pool.tile([128, NCHUNK, C], bf16, tag="ea")
            bounds = [0, 1] + list(range(1 + DMA_BATCH, NCHUNK, DMA_BATCH)) + [NCHUNK]
            for b0, b1 in zip(bounds[:-1], bounds[1:]):
                nc.sync.dma_start(lh_all[:, b0:b1, :], lh_v[:, b0:b1, :])
                nc.sync.dma_start(e_all[:, b0:b1, :], e_v[:, b0:b1, :])

            # per-shard per-bin stats: cols [0:10]=S1, [10:20]=SE, [20:30]=S2, [30:40]=T
            stat = small_pool.tile([128, 4 * NCHUNK], f32, tag="stat")

            for a in range(NCHUNK):
                g_t = scr_pool.tile([128, C], f32, tag="g")
                s2_t = scr_pool.tile([128, C], f32, tag="d2")
                ge_t = scr_pool.tile([128, C], f32, tag="d3")
                # S1 = sum exp(lh); keeps g for the T pass
                nc.scalar.activation(
                    out=g_t[:], in_=lh_all[:, a, :], func=Act.Exp,
                    accum_out=stat[:, a : a + 1])
                # S2 = sum exp(2*lh) == sum exp(lh)^2
                nc.scalar.activation(
                    out=s2_t[:], in_=lh_all[:, a, :], func=Act.Exp, scale=2.0,
                    accum_out=stat[:, 2 * NCHUNK + a : 2 * NCHUNK + a + 1])
                # T = sum exp(lh)*e  (fused multiply + row-accumulate)
                nc.vector.scalar_tensor_tensor(
                    out=ge_t[:], in0=g_t[:], scalar=1.0, in1=e_all[:, a, :],
                    op0=Alu.mult, op1=Alu.mult,
                    accum_out=stat[:, 3 * NCHUNK + a : 3 * NCHUNK + a + 1])
                # SE = sum(e) per row
                nc.vector.tensor_reduce(
                    out=stat[:, NCHUNK + a : NCHUNK + a + 1],
                    in_=e_all[:, a, :], axis=Ax.X, op=Alu.add)

            # constants to SBUF (issued late; only needed after the collective)
            tril_inc_t = small_pool.tile([128, 128], f32, tag="c0")
            tril_str_t = small_pool.tile([128, 128], f32, tag="c1")
            allones_t = small_pool.tile([128, 128], f32, tag="c2")
            ones_t = small_pool.tile([128, 1], f32, tag="c3")
            nc.sync.dma_start(tril_inc_t[:], tril_inc_h.ap())
            nc.sync.dma_start(tril_str_t[:], tril_str_h.ap())
            nc.sync.dma_start(allones_t[:], allones_h.ap())
            nc.sync.dma_start(ones_t[:], ones_h.ap())

            # ---- exchange per-bin stats across all cores ----
            cc_in = dram_pool.tile([128, 4 * NCHUNK], f32)
            cc_out = dram_pool.tile([128 * NCORES, 4 * NCHUNK], f32,
                                    addr_space="Shared")
            nc.sync.dma_start(cc_in[:], stat[:])
            nc.gpsimd.collective_compute(
                "AllGather",
                Alu.bypass,
                replica_groups=[list(range(NCORES))],
                ins=[cc_in.opt()],
                outs=[cc_out.opt()],
            )
            # [128, s, q]: global (padded) bin index beta = s*1280 + a*128 + p
            allstat = small_pool.tile([128, NCORES * 4 * NCHUNK], f32, tag="all")
            av = allstat[:].rearrange("p (s q) -> p s q", s=NCORES)
            cc_v = cc_out.opt().rearrange("(s p) q -> p s q", p=128)
            nc.sync.dma_start(av[:, :, 0:NCHUNK], cc_v[:, :, 0:NCHUNK])
            nc.sync.dma_start(av[:, :, NCHUNK:4 * NCHUNK],
                              cc_v[:, :, NCHUNK:4 * NCHUNK])
            NCOL = NCORES * NCHUNK  # 80 (s-major, then chunk) columns per quantity
            v3 = lambda t: t[:].rearrange("p (s q) -> p s q", s=NCORES)

            # ---- risk = suffix-cumsum of S1 over the global bin order ----
            # within-column (partition-axis) inclusive suffix sums
            s1c = small_pool.tile([128, NCOL], f32, tag="s1c")
            nc.vector.tensor_copy(out=v3(s1c), in_=av[:, :, 0:NCHUNK])
            cw_ps = psum_pool.tile([128, NCOL], f32, space="PSUM", tag="cw")
            nc.tensor.matmul(out=cw_ps[:], lhsT=tril_inc_t[:],
                             rhs=av[:, :, 0:NCHUNK], start=True, stop=True)
            cws = small_pool.tile([128, NCOL], f32, tag="cws")
            nc.vector.tensor_copy(out=cws[:], in_=cw_ps[:])
            # column totals as a [NCOL,1] column vector (S1cols^T @ ones)
            totT_ps = psum_pool.tile([NCOL, 1], f32, space="PSUM", tag="tt")
            nc.tensor.matmul(out=totT_ps[:], lhsT=s1c[:],
                             rhs=ones_t[:], start=True, stop=True)
            totT = small_pool.tile([128, 1], f32, tag="totT")
            nc.vector.memset(totT[:], 0.0)
            nc.vector.tensor_copy(out=totT[0:NCOL, :], in_=totT_ps[:])
            # rr[c',c] = tot[c'] * [c' > c]; column-sum it with an all-ones
            # matmul to broadcast the exclusive column-suffix to every row
            rr = small_pool.tile([128, NCOL], f32, tag="rr")
            nc.vector.tensor_tensor(
                out=rr[:], in0=tril_str_t[:, 0:NCOL],
                in1=totT[:, 0:1].to_broadcast([128, NCOL]), op=Alu.mult)
            offbc_ps = psum_pool.tile([128, NCOL], f32, space="PSUM", tag="cw")
            nc.tensor.matmul(out=offbc_ps[:], lhsT=allones_t[:],
                             rhs=rr[:], start=True, stop=True)
            risk = small_pool.tile([128, NCOL], f32, tag="risk")
            nc.vector.tensor_tensor(
                out=risk[:], in0=cws[:], in1=offbc_ps[:], op=Alu.add)

            # base = ev_sum / risk  (0 where risk == 0, since then ev_sum == 0;
            # the epsilon keeps the trailing all-pad rows NaN-free)
            nc.vector.tensor_scalar_max(risk[:], risk[:], 1e-30)
            rrec = small_pool.tile([128, NCOL], f32, tag="rrec")
            nc.vector.reciprocal(rrec[:], risk[:])
            base = small_pool.tile([128, NCOL], f32, tag="base")
            nc.vector.tensor_tensor(
                out=v3(base), in0=av[:, :, NCHUNK:2 * NCHUNK],
                in1=v3(rrec), op=Alu.mult)

            # mse*N = sum(base * (base*S2 - 2*T)) + E
            t1 = small_pool.tile([128, NCOL], f32, tag="t1")
            nc.vector.tensor_tensor(
                out=v3(t1), in0=av[:, :, 2 * NCHUNK:3 * NCHUNK],
                in1=v3(base), op=Alu.mult)
            t2 = small_pool.tile([128, NCOL], f32, tag="t2")
            nc.vector.scalar_tensor_tensor(
                out=v3(t2), in0=av[:, :, 3 * NCHUNK:4 * NCHUNK], scalar=-2.0,
                in1=v3(t1), op0=Alu.mult, op1=Alu.add)
            finvec = small_pool.tile([128, 2], f32, tag="finvec")
            vtile = small_pool.tile([128, NCOL], f32, tag="vtile")
            nc.vector.scalar_tensor_tensor(
                out=vtile[:], in0=base[:], scalar=1.0, in1=t2[:],
                op0=Alu.mult, op1=Alu.mult, accum_out=finvec[:, 0:1])
            nc.vector.tensor_reduce(
                out=finvec[:, 1:2], in_=av[:, :, NCHUNK:2 * NCHUNK],
                axis=Ax.XY, op=Alu.add)

            vE = small_pool.tile([128, 1], f32, tag="vE")
            nc.vector.tensor_tensor(out=vE[:], in0=finvec[:, 0:1],
                                    in1=finvec[:, 1:2], op=Alu.add)
            fin_ps = psum_pool.tile([1, 1], f32, space="PSUM", tag="fin")
            nc.tensor.matmul(out=fin_ps[:], lhsT=ones_t[:], rhs=vE[:],
                             start=True, stop=True)
            mse_t = small_pool.tile([1, 1], f32, tag="mse")
            nc.vector.tensor_scalar_mul(mse_t[:], fin_ps[0:1, 0:1], 1.0 / N)
            nc.sync.dma_start(mse_d.ap(), mse_t[:])

    nc.compile()
    return nc


def _shard_inputs(log_h, durations, events, C):
    """Host-side sharding: counting-layout [NCORES*R, C] dense matrices."""
    import ml_dtypes

    d = np.ascontiguousarray(durations.astype(np.int64, copy=False))
    order = np.argsort(d, kind="stable")
    d_sorted = d[order]
    counts = np.bincount(d, minlength=K)
    starts = np.zeros(K, np.int64)
    starts[1:] = np.cumsum(counts)[:-1]
    slot = np.arange(N, dtype=np.int64) - starts[d_sorted]
    assert slot.max() < C, f"bin count {slot.max() + 1} exceeds capacity {C}"
    rows = (d_sorted // BINS_PER_SHARD) * R + (d_sorted % BINS_PER_SHARD)

    bf = ml_dtypes.bfloat16
    lh_dense = np.full((NCORES * R, C), PAD_LH, dtype=bf)
    e_dense = np.zeros((NCORES * R, C), dtype=bf)
    lh_dense[rows, slot] = log_h[order].astype(bf)
    e_dense[rows, slot] = events[order].astype(bf)

    in_maps = []
    for s in range(NCORES):
        in_maps.append({
            "lh_d": np.ascontiguousarray(lh_dense[s * R:(s + 1) * R]),
            "e_d": np.ascontiguousarray(e_dense[s * R:(s + 1) * R]),
        })
    return in_maps


def kernel(log_h, durations, events):
    global LAST_EXEC_TIME_NS, LAST_RESULTS
    from concourse.bass_utils import run_bass_kernel_spmd

    assert log_h.shape == (N,) and durations.shape == (N,)

    counts_max = int(np.bincount(durations.astype(np.int64), minlength=K).max())
    C = 928 if counts_max <= 928 else ((counts_max + 127) // 128) * 128

    if C not in _CACHE:
        _CACHE[C] = _build_program(C)
    nc = _CACHE[C]

    in_maps = _shard_inputs(log_h, durations, events, C)
    tc_env = os.environ.get("KERNEL_TRACE_CORES", "")
    trace_cores = [int(x) for x in tc_env.split(",") if x] or None
    res = run_bass_kernel_spmd(
        nc, in_maps, core_ids=list(range(NCORES)), trace=TRACE,
        trace_cores=trace_cores)
    LAST_EXEC_TIME_NS = res.exec_time_ns
    LAST_RESULTS = res
    mse = res.results[0]["mse_d"][0, 0]
    return np.asarray(mse, dtype=np.float32).reshape(())
